# revision 1
# baseline (speedup 1.0000x reference)
"""Trainium2 Bass kernel for nn_DebugQuantizedLinear.

Computes out = x @ W_deq.T where
  W_deq = ((W_q - zeros) * scales).reshape(K, N) * mu2[:, None] * mu1[None, :]
  x: [B, N] f32, W_q: [K, N] int32 (values 0..15), out: [B, K] f32
  K=11008, N=4096, B=8192, group size 64 along N (NG=64 groups).

Strategy (8 NeuronCores, tensor-parallel along K):
  - K padded 11008 -> 11264 = 8 * 1408; core c owns rows [c*1408, (c+1)*1408).
  - Host supplies x transposed (xT [N, B] f32, replicated) so the contraction
    dim N lands on SBUF partitions for both matmul operands.
  - Phase 1 (per core, once): stream W_q shard k-tile by k-tile, dequantize in
    natural [k, n] layout with fused (Q - z) * (s * mu2) tensor_scalar ops
    (per-partition scalars) to fp16, PE-transpose each 128x128 block, and
    scale by mu1 during the PSUM drain into an SBUF-resident fp16
    W^T [N, 1408].
  - Phase 2: stream xT in 512-column half-panels (cast f32->fp16 by DMA),
    accumulate out^T tiles [128 k, 512 b] in PSUM over the 32 n-tiles,
    drain to SBUF, DMA to DRAM outT [1408, B] f32.
  - Host assembles out[B, K] from the 8 outT shards (transpose + concat).

fp16 weights/activations with fp32 PSUM accumulation give ~3e-4 relative
error vs the f32 reference (measured on the real data distribution).
"""

import os
from contextlib import ExitStack

import numpy as np

K, N, B = 11008, 4096, 8192
GROUP = 64
NG = N // GROUP
NCORES = 8
KC = 1408               # per-core padded K rows
KPAD = KC * NCORES      # 11264
P = 128

_PROGRAM_CACHE = {}
LAST_RESULTS = None     # BassKernelResults of the most recent run (for test.py)


def _build_program(kc=KC, b=B, bh=512, x_cast_dma=True):
    """Build the SPMD Bass program (identical on all cores)."""
    import concourse.bacc as bacc
    import concourse.bass as bass
    import concourse.mybir as mybir
    from concourse.tile import TileContext

    f32 = mybir.dt.float32
    f16 = mybir.dt.float16
    i32 = mybir.dt.int32

    nkt = kc // P           # k-tiles per core
    nnt = N // P            # 32 n-tiles
    nh = b // bh            # half-panels
    sub = mybir.AluOpType.subtract
    mul = mybir.AluOpType.mult

    nc = bacc.Bacc(num_swdge_queues=4)
    xT = nc.declare_dram_parameter("xT", [N, b], f32, isOutput=False)
    wq = nc.declare_dram_parameter("wq", [kc, N], i32, isOutput=False)
    zr = nc.declare_dram_parameter("zr", [P, nkt * NG], f32, isOutput=False)
    sc = nc.declare_dram_parameter("sc", [P, nkt * NG], f32, isOutput=False)
    mu1 = nc.declare_dram_parameter("mu1", [P, nnt], f32, isOutput=False)
    mu2 = nc.declare_dram_parameter("mu2", [P, nkt], f32, isOutput=False)
    ident = nc.declare_dram_parameter("ident", [P, P], f16, isOutput=False)
    outT = nc.declare_dram_parameter("outT", [kc, b], f32, isOutput=True)

    with TileContext(nc) as tc, ExitStack() as ctx:
        const = ctx.enter_context(tc.tile_pool(name="const", bufs=1))
        ident_t = const.tile([P, P], f16, name="ident_t")
        nc.sync.dma_start(out=ident_t[:, :], in_=ident[:, :])
        mu1_t = const.tile([P, nnt], f32, name="mu1_t")
        nc.sync.dma_start(out=mu1_t[:, :], in_=mu1[:, :])
        mu2_t = const.tile([P, nkt], f32, name="mu2_t")
        nc.sync.dma_start(out=mu2_t[:, :], in_=mu2[:, :])
        zr_t = const.tile([P, nkt, NG], f32, name="zr_t")
        nc.sync.dma_start(out=zr_t[:, :, :], in_=zr[:, :])
        sc_t = const.tile([P, nkt, NG], f32, name="sc_t")
        nc.sync.dma_start(out=sc_t[:, :, :], in_=sc[:, :])
        sp_t = const.tile([P, nkt, NG], f32, name="sp_t")

        # SBUF-resident transposed dequantized weights, one tile per k-tile:
        # [128 n-partitions, n_tile, 128 k] fp16.
        wdqT = [const.tile([P, nnt, P], f16, name=f"wdqT_{kt}") for kt in range(nkt)]

        wqpool = ctx.enter_context(tc.tile_pool(name="wqpool", bufs=2))
        wdqpool = ctx.enter_context(tc.tile_pool(name="wdqpool", bufs=4))
        tpsum = ctx.enter_context(tc.tile_pool(name="tpsum", bufs=2, space="PSUM"))
        xpool = ctx.enter_context(tc.tile_pool(name="xpool", bufs=2))
        opsum = ctx.enter_context(tc.tile_pool(name="opsum", bufs=6, space="PSUM"))
        opool = ctx.enter_context(tc.tile_pool(name="opool", bufs=3))

        def load_x_half(h):
            xh = xpool.tile([P, nnt, bh], f16, name="xh")
            src = xT[:, h * bh:(h + 1) * bh].rearrange("(t p) b -> p t b", p=P)
            step = nnt // 4
            for q in range(4):
                sl = slice(q * step, (q + 1) * step)
                if x_cast_dma:
                    nc.gpsimd.dma_start(out=xh[:, sl, :], in_=src[:, sl, :])
                else:
                    stage = xpool.tile([P, step, bh], f32, name="xstage")
                    nc.sync.dma_start(out=stage[:, :, :], in_=src[:, sl, :])
                    nc.vector.tensor_copy(xh[:, sl, :], stage[:, :, :])
            return xh

        def phase1_ktile(kt):
            """Dequantize + transpose k-tile kt into wdqT[kt]."""
            nc.vector.tensor_scalar_mul(
                sp_t[:, kt, :], sc_t[:, kt, :], mu2_t[:, kt:kt + 1])
            wq_t = wqpool.tile([P, N], i32, name="wq_t")
            qn = N // 4
            for q in range(4):
                nc.sync.dma_start(
                    out=wq_t[:, q * qn:(q + 1) * qn],
                    in_=wq[kt * P:(kt + 1) * P, q * qn:(q + 1) * qn])
            for nt in range(nnt):
                wdq_t = wdqpool.tile([P, P], f16, name="wdq_t")
                for gi in range(P // GROUP):
                    g = nt * (P // GROUP) + gi
                    nc.vector.tensor_scalar(
                        out=wdq_t[:, gi * GROUP:(gi + 1) * GROUP],
                        in0=wq_t[:, nt * P + gi * GROUP: nt * P + (gi + 1) * GROUP],
                        scalar1=zr_t[:, kt, g:g + 1],
                        scalar2=sp_t[:, kt, g:g + 1],
                        op0=sub, op1=mul)
                # PE transpose on fp16 (FWL-eligible weight load, ~2x
                # cheaper than fp32); fold mu1 into the PSUM->SBUF copy.
                ps = tpsum.tile([P, P], f16, name="tps")
                nc.tensor.transpose(ps[:, :], wdq_t[:, :], ident_t[:, :])
                nc.scalar.mul(wdqT[kt][:, nt, :], ps[:, :], mu1_t[:, nt:nt + 1])

        def matmuls(h, kt, xh):
            ps = opsum.tile([P, bh], f32, name="ops")
            for nt in range(nnt):
                nc.tensor.matmul(
                    ps[:, :],
                    lhsT=wdqT[kt][:, nt, :],
                    rhs=xh[:, nt, :],
                    start=(nt == 0), stop=(nt == nnt - 1))
            ot = opool.tile([P, bh], f32, name="ot")
            nc.scalar.copy(ot[:, :], ps[:, :])
            nc.sync.dma_start(
                out=outT[kt * P:(kt + 1) * P, h * bh:(h + 1) * bh], in_=ot[:, :])

        # Interleave: h=0 matmuls ride along with phase 1 so the PE never
        # idles waiting for all weights; h>=1 are pure matmul sweeps.
        # kt=0's weight pipeline is emitted before the x load so the first
        # matmul isn't gated on both arriving serially.
        phase1_ktile(0)
        xh = load_x_half(0)
        matmuls(0, 0, xh)
        for kt in range(1, nkt):
            phase1_ktile(kt)
            matmuls(0, kt, xh)
        for h in range(1, nh):
            xh = load_x_half(h)
            for kt in range(nkt):
                matmuls(h, kt, xh)

    # Run Bacc's compile passes (register allocation, sync-wait splitting
    # into EventSemaphores, nop fusion). The axon/PJRT exec path serializes
    # the module as-is, so finalize here.
    nc.finalize()
    return nc


def _get_program(key=()):
    if key not in _PROGRAM_CACHE:
        _PROGRAM_CACHE[key] = _build_program(*key) if key else _build_program()
    return _PROGRAM_CACHE[key]


def kernel(x, W_q, zeros, scales, mu1, mu2):
    global LAST_RESULTS
    from concourse.bass_utils import run_bass_kernel_spmd

    x = np.asarray(x)
    W_q = np.asarray(W_q)
    zeros = np.asarray(zeros)
    scales = np.asarray(scales)
    mu1 = np.asarray(mu1)
    mu2 = np.asarray(mu2)

    # Host-side layout prep (no arithmetic): transpose x, pad K to 8*1408.
    NKT = KC // P
    xT = np.ascontiguousarray(x.T)                      # [N, B] f32
    wq_p = np.zeros((KPAD, N), dtype=W_q.dtype)
    wq_p[:K] = W_q
    zr_p = np.zeros((KPAD, NG), dtype=zeros.dtype)
    zr_p[:K] = zeros.reshape(K, NG)
    sc_p = np.zeros((KPAD, NG), dtype=scales.dtype)
    sc_p[:K] = scales.reshape(K, NG)
    mu2_p = np.zeros((KPAD,), dtype=mu2.dtype)
    mu2_p[:K] = mu2
    mu1_r = np.ascontiguousarray(mu1.reshape(N // P, P).T)      # [128, nnt]

    def part_major(a2d):
        # [KC, G] -> [128, NKT*G], partition-major for a clean DMA
        g = a2d.shape[1]
        return np.ascontiguousarray(
            a2d.reshape(NKT, P, g).transpose(1, 0, 2).reshape(P, NKT * g))
    ident = np.eye(P, dtype=np.float16)
    in_maps = []
    for c in range(NCORES):
        lo, hi = c * KC, (c + 1) * KC
        in_maps.append({
            "xT": xT,
            "wq": np.ascontiguousarray(wq_p[lo:hi]),
            "zr": part_major(zr_p[lo:hi]),
            "sc": part_major(sc_p[lo:hi]),
            "mu1": mu1_r,
            "mu2": np.ascontiguousarray(mu2_p[lo:hi].reshape(NKT, P).T),
            "ident": ident,
        })

    nc = _get_program()
    trace = bool(os.environ.get("KERNEL_TRACE"))
    res = run_bass_kernel_spmd(nc, in_maps, list(range(NCORES)), trace=trace)
    LAST_RESULTS = res

    out = np.empty((B, K), dtype=np.float32)
    for c in range(NCORES):
        lo = c * KC
        hi = min(lo + KC, K)
        out[:, lo:hi] = res.results[c]["outT"][:hi - lo].T
    return out

